# revision 25
# baseline (speedup 1.0000x reference)
"""Trainium2 Bass kernel for the CycleConsistency formant-extraction pipeline.

Pipeline per frame (64*1000 = 64000 independent frames):
  reflection coeffs (10) -> step-up recursion -> predictor poly A (11)
  -> power spectrum at 116 rfft bins -> autocorrelation (11 lags)
  -> Levinson-Durbin -> monic allpole poly (11)
  -> Durand-Kerner (clamped) -> 10 roots
  -> angles -> validity mask -> partial sort -> lowest 4 -> normalize

Sharding: batch dim across 8 cores (8 batches/core = 8000 frames/core,
padded to 8064 = 128*63).  SoA layout: per-frame scalar = [128, 63] tile
(frame = partition*63 + col); Durand-Kerner state = [128, 630]
(10 root-major blocks of 63).

v8: mixed-precision Durand-Kerner + DMA layout fixes.
  HW-measured per-[128,630]-op DVE costs: tensor_tensor fp32 ~577ns,
  bf16 ~363ns (2x_1p), scalar_tensor_tensor ~629ns regardless of dtype
  (no DVE perf mode), tensor_scalar imm fp32 ~381 / bf16 ~180 (2x_2p /
  4x_2p).  So the DK loop is emitted as plain tensor_tensor ops, runs
  14 iterations in bf16 and 4 in fp32 (HW rel err 1.34e-2 vs the 2e-2
  gate).  The denominator eps guard is a ts-add of 1e-30 before the
  reciprocal, keeping every op a plain TT/TS with no NaN paths.
  Coefficients are read through stride-0 broadcast APs (no block
  broadcast copies).  kin/out DRAM layouts are per-partition contiguous
  and the two transposing stage-A DMAs are split across the three
  DMA-capable queues (SP/ACT/Pool) and pipelined chunk-wise with the
  spectrum matmuls.
"""

import numpy as np

# ---------------------------------------------------------------- constants
B, P, T = 64, 10, 1000
NCORES = 8
BPC = B // NCORES            # batches per core
FPC = BPC * T                # frames per core (8000)
PART = 128
WCOL = 63                    # columns per SoA tile
FPAD = PART * WCOL           # padded frames per core (8064)
NROOT = P
WBIG = WCOL * NROOT          # 630
WMIR = WBIG + 9 * WCOL       # 1197: mirror-extended x tiles
NF = 116                     # spectrum bins
NCHUNK = 16
CH = FPAD // NCHUNK          # 504 (one psum bank)
N_IT_BF = 15                 # bf16 DK iterations
N_IT_F32 = 4                 # fp32 DK iterations
Q_CLAMP = 2.0
RMAG_CLAMP = 1e30

FM_SR = 10000.0
RC_SR = 22050.0
ANG_LO = np.float32(50.0 * 2.0 * np.pi / FM_SR)
ANG_HI = np.float32((FM_SR / 2 - 50.0) * 2.0 * np.pi / FM_SR)
ANG_INVALID = np.float32(2.0 * np.pi)
OUT_SCALE = np.float32((FM_SR / (2.0 * np.pi)) * 2.0 / (RC_SR / 2.0))
PI = np.float32(np.pi)

_DK_INIT = ((0.4 + 0.9j) ** np.arange(1, P + 1)).astype(np.complex64)


def _spec_consts():
    j = np.arange(P + 1)[:, None]
    k = np.arange(NF)[None, :]
    C = np.cos(2 * np.pi * j * k / 512.0).astype(np.float32)   # [11, 116]
    S = np.sin(2 * np.pi * j * k / 512.0).astype(np.float32)   # [11, 116]
    kk = np.arange(NF)[:, None]
    m = np.arange(P + 1)[None, :]
    cc = np.full((NF, 1), 2.0)
    cc[0] = 1.0
    cc[NF - 1] = 1.0
    W = ((1.0 / 230.0) * cc * np.cos(2 * np.pi * kk * m / 230.0)).astype(np.float32)
    return np.concatenate([C, S], axis=1), W   # [11, 232], [116, 11]


# ------------------------------------------------------- tile workarounds
def _install_tile_patches():
    import bass_rust
    import concourse.tile as tile
    from concourse.vector_clock import ScopedClock

    if getattr(tile.TileContext, "_drain_patched", False):
        return

    def _drain_and_barrier(self, tick_clock, wait_clock):
        # this walrus build accepts only ONE sync-wait command per
        # instruction; fan the tail-drain waits out over NOPs.
        gc = tick_clock.global_clock
        n = len(gc)
        for i in [i for i in range(n) if gc[i] > 0]:
            partial = bass_rust.VectorClock(
                [gc[j] if j == i else 0 for j in range(n)]
            )
            nop = self.nc.sync.nop()
            wait_clock.add_sem_waits(nop.ins, ScopedClock({None: partial}))
        self.nc.sync.drain()
        self.nc.all_engine_barrier()
        popped = self.nc._tile_sem_poison_stack.pop()
        assert popped is self._sem_poison
        self.nc.clear_and_free_semaphores(list(self.sems.allocated().values()))
        self.nc.all_engine_barrier()

    tile.TileContext._drain_and_barrier = _drain_and_barrier
    tile.TileContext._drain_patched = True


def _split_multi_waits(nc):
    import concourse.mybir as mybir

    ctr = 0
    for func in nc.m.functions:
        for bb in func.blocks:
            out = []
            for ins in bb.instructions:
                si = ins.sync_info
                if si is not None and si.on_wait is not None and len(si.on_wait) > 1:
                    waits = list(si.on_wait)
                    for w in waits[:-1]:
                        nop = mybir.InstNoOp(name=f"I-ws{ctr}")
                        ctr += 1
                        nop.engine = ins.engine
                        nop.sync_info = mybir.SyncInfo(on_wait=[w], on_update=[])
                        out.append(nop)
                    ins.sync_info = mybir.SyncInfo(
                        on_wait=[waits[-1]],
                        on_update=list(si.on_update) if si.on_update else [],
                    )
                out.append(ins)
            bb.instructions[:] = out


# ------------------------------------------------------------- bass module
def _build_module():
    import concourse.bass as bass
    import concourse.mybir as mybir
    import concourse.tile as tile

    _install_tile_patches()

    F32 = mybir.dt.float32
    BF16 = mybir.dt.bfloat16
    U8 = mybir.dt.uint8
    Alu = mybir.AluOpType
    Act = mybir.ActivationFunctionType

    nc = bass.Bass()
    # kin rows are per-partition contiguous (p, (c,w)) so the load is one
    # DMA with 2520B descriptors instead of 1280 x 252B
    kin = nc.dram_tensor("kin", [PART, P * WCOL], F32, kind="ExternalInput")
    cs_d = nc.dram_tensor("cs", [P + 1, 2 * NF], F32, kind="ExternalInput")
    wm_d = nc.dram_tensor("wm", [NF, P + 1], F32, kind="ExternalInput")
    out_d = nc.dram_tensor("out", [PART, 4 * WCOL], F32, kind="ExternalOutput")
    alay_d = nc.dram_tensor("alay", [P + 1, FPAD], F32)
    rlay_d = nc.dram_tensor("rlay", [P + 1, FPAD], F32)

    lowp = nc.allow_low_precision(reason="bf16 DK phase; validated vs 2e-2 gate")
    lowp.__enter__()

    with tile.TileContext(nc) as tc:
        with tc.tile_pool(name="persist", bufs=1) as pp:

            # STT helpers (stage A, [128,63] tiles where fusion still wins).
            def stt(out, a, b, op1, s=1.0, op0=Alu.mult, eng=None):
                (eng or nc.vector).scalar_tensor_tensor(out, a, float(s), b, op0, op1)

            # plain tensor_tensor emitters (stage B workhorses)
            def vmul(out, a, b):      # out = a*b
                nc.vector.tensor_tensor(out, a, b, Alu.mult)

            def vadd(out, a, b):      # out = a+b
                nc.vector.tensor_tensor(out, a, b, Alu.add)

            def vsub(out, a, b):      # out = a-b
                nc.vector.tensor_tensor(out, a, b, Alu.subtract)

            def vrsub(out, a, b):     # out = b-a
                nc.vector.tensor_tensor(out, b, a, Alu.subtract)

            # persistent LPC coefficients: one [128,63] tile per order;
            # the DK loop reads them through stride-0 broadcast APs
            laP = [pp.tile([PART, WCOL], F32, tag=f"lap{j}", name=f"lap{j}")
                   for j in range(P)]
            la16 = [pp.tile([PART, WCOL], BF16, tag=f"la16_{j}", name=f"la16_{j}")
                    for j in range(P)]
            # DK state: bf16 phase-1 tiles and fp32 phase-2 tiles
            xrm16 = pp.tile([PART, WMIR], BF16, tag="xrm16", name="xrm16")
            xim16 = pp.tile([PART, WMIR], BF16, tag="xim16", name="xim16")
            xrm = pp.tile([PART, WMIR], F32, tag="xrm", name="xrm")
            xim = pp.tile([PART, WMIR], F32, tag="xim", name="xim")

            # ============ stage A: everything before Durand-Kerner ============
            with tc.tile_pool(name="pre", bufs=1) as prep, \
                 tc.tile_pool(name="pret", bufs=2) as pret, \
                 tc.tile_pool(name="psum", bufs=2, space="PSUM") as psp:

                # ---- load K (single contiguous DMA), forward levinson ----
                ka = prep.tile([PART, P * WCOL], F32, tag="ka", name="ka")
                nc.sync.dma_start(out=ka[:], in_=kin[:])
                kt = [ka[:, p_ * WCOL:(p_ + 1) * WCOL] for p_ in range(P)]

                a_all = prep.tile([PART, P * WCOL], F32, tag="a_all", name="a_all")
                a = [kt[0]]            # list of APs
                for p_ in range(1, P):
                    kp = kt[p_]
                    na = []
                    for i in range(p_):
                        prod = pret.tile([PART, WCOL], F32, tag="fl_prod", name="fl_prod")
                        vmul(prod[:], kp, a[p_ - 1 - i])
                        if p_ == P - 1:
                            s = a_all[:, i * WCOL:(i + 1) * WCOL]
                        else:
                            s = prep.tile([PART, WCOL], F32, tag=f"a{p_}_{i}", name=f"a{p_}_{i}")[:]
                        vadd(s, a[i], prod[:])
                        na.append(s)
                    na.append(kp)
                    a = na
                nc.scalar.copy(a_all[:, (P - 1) * WCOL:], kt[P - 1])

                # transposing DMA a_all -> alay rows 1..10 has 252B
                # descriptors; split across 4 DGE queues by partition range
                qsplit = [(nc.sync, slice(0, 43)), (nc.scalar, slice(43, 86)),
                          (nc.gpsimd, slice(86, PART))]
                for qeng, pr in qsplit:
                    qeng.dma_start(
                        out=alay_d[1:].rearrange("c (p w) -> p c w",
                                                 p=PART)[pr],
                        in_=a_all[pr].rearrange("p (c w) -> p c w", w=WCOL),
                    )
                A_lay = prep.tile([P + 1, FPAD], F32, tag="A_lay", name="A_lay")
                nc.vector.memset(A_lay[0:1, :], 1.0)
                for ch in range(NCHUNK):
                    sl = slice(ch * CH, (ch + 1) * CH)
                    nc.sync.dma_start(out=A_lay[1:, sl], in_=alay_d[1:, sl])

                # ---- spectrum + autocorrelation (TensorE matmuls) ----
                cs = prep.tile([P + 1, 2 * NF], F32, tag="cs", name="cs")
                nc.sync.dma_start(out=cs[:], in_=cs_d[:])
                wm = prep.tile([NF, P + 1], F32, tag="wm", name="wm")
                nc.sync.dma_start(out=wm[:], in_=wm_d[:])
                r_lay = prep.tile([P + 1, FPAD], F32, tag="r_lay", name="r_lay")

                for ch in range(NCHUNK):
                    sl = slice(ch * CH, (ch + 1) * CH)
                    ps_re = psp.tile([NF, CH], F32, tag="ps_re", name="ps_re")
                    ps_im = psp.tile([NF, CH], F32, tag="ps_im", name="ps_im")
                    nc.tensor.matmul(ps_re[:], cs[:, 0:NF], A_lay[:, sl], start=True, stop=True)
                    nc.tensor.matmul(ps_im[:], cs[:, NF:2 * NF], A_lay[:, sl], start=True, stop=True)
                    sq_re = pret.tile([NF, CH], F32, tag="sq_re", name="sq_re")
                    sq_im = pret.tile([NF, CH], F32, tag="sq_im", name="sq_im")
                    nc.scalar.activation(sq_re[:], ps_re[:], Act.Square)
                    nc.scalar.activation(sq_im[:], ps_im[:], Act.Square)
                    spec = pret.tile([NF, CH], F32, tag="spec", name="spec")
                    vadd(spec[:], sq_re[:], sq_im[:])
                    ps_r = psp.tile([P + 1, CH], F32, tag="ps_r", name="ps_r")
                    nc.tensor.matmul(ps_r[:], wm[:], spec[:], start=True, stop=True)
                    nc.vector.tensor_copy(r_lay[:, sl], ps_r[:])

                r_all = prep.tile([PART, (P + 1) * WCOL], F32, tag="r_all", name="r_all")
                qcycle = [nc.sync, nc.scalar, nc.gpsimd]
                for ch in range(NCHUNK):
                    sl = slice(ch * CH, (ch + 1) * CH)
                    nc.sync.dma_start(out=rlay_d[:, sl], in_=r_lay[:, sl])
                    band = slice(ch * (PART // NCHUNK), (ch + 1) * (PART // NCHUNK))
                    qcycle[ch % 3].dma_start(
                        out=r_all[band].rearrange("p (c w) -> p c w", w=WCOL),
                        in_=rlay_d.rearrange("c (p w) -> p c w", p=PART)[band],
                    )
                r = [r_all[:, m_ * WCOL:(m_ + 1) * WCOL] for m_ in range(P + 1)]

                # ---- Levinson-Durbin (SoA; everything passed as APs) ----
                def div_newton(num, den, tag, negate=False):
                    """q = (-)num/den via DVE reciprocal (shorter serial
                    chain; DVE recip is accurate enough for the LD k's)."""
                    rc_ = pret.tile([PART, WCOL], F32, tag="ldv_rc", name="ldv_rc")
                    nc.vector.reciprocal(rc_[:], den)
                    q = prep.tile([PART, WCOL], F32, tag=tag, name=tag)
                    if negate:
                        stt(q[:], num, rc_[:], Alu.mult, s=-1.0)
                    else:
                        vmul(q[:], num, rc_[:])
                    return q[:]

                k0 = div_newton(r[1], r[0], "ld_k0", negate=True)
                la = [k0]
                err = prep.tile([PART, WCOL], F32, tag="ld_err", name="ld_err")
                om = pret.tile([PART, WCOL], F32, tag="ld_om", name="ld_om")
                ksq = pret.tile([PART, WCOL], F32, tag="ld_ksq", name="ld_ksq")
                stt(ksq[:], k0, k0, Alu.mult, s=-1.0)   # -k0^2
                nc.vector.tensor_scalar(om[:], ksq[:], 1.0, None, Alu.add)
                vmul(err[:], r[0], om[:])
                for m_ in range(1, P):
                    # products in parallel, then a pairwise tree reduction
                    # (depth log m instead of m serial adds)
                    terms = [r[m_ + 1]]
                    for i in range(m_):
                        prd = pret.tile([PART, WCOL], F32, tag=f"ld_p{m_}_{i}",
                                        name=f"ld_p{m_}_{i}")
                        vmul(prd[:], la[i], r[m_ - i])
                        terms.append(prd[:])
                    lvl = 0
                    while len(terms) > 1:
                        nt = []
                        for j in range(0, len(terms) - 1, 2):
                            t_ = pret.tile([PART, WCOL], F32,
                                           tag=f"ld_tr{m_}_{lvl}_{j}",
                                           name=f"ld_tr{m_}_{lvl}_{j}")
                            vadd(t_[:], terms[j], terms[j + 1])
                            nt.append(t_[:])
                        if len(terms) % 2:
                            nt.append(terms[-1])
                        terms = nt
                        lvl += 1
                    kk = div_newton(terms[0], err[:], f"ld_k{m_}", negate=True)
                    nla = []
                    for i in range(m_):
                        prd = pret.tile([PART, WCOL], F32, tag="ld_p2", name="ld_p2")
                        vmul(prd[:], kk, la[m_ - 1 - i])
                        s = prep.tile([PART, WCOL], F32, tag=f"c{m_}_{i}", name=f"c{m_}_{i}")
                        vadd(s[:], la[i], prd[:])
                        nla.append(s[:])
                    nla.append(kk)
                    la = nla
                    if m_ < P - 1:
                        ksq2 = pret.tile([PART, WCOL], F32, tag="ld_ksq2", name="ld_ksq2")
                        stt(ksq2[:], kk, kk, Alu.mult, s=-1.0)   # -k^2
                        om2 = pret.tile([PART, WCOL], F32, tag="ld_om2", name="ld_om2")
                        nc.vector.tensor_scalar(om2[:], ksq2[:], 1.0, None, Alu.add)
                        vmul(err[:], err[:], om2[:])

                # persist coefficients (fp32 + bf16 conversion copies)
                for j in range(P):
                    nc.vector.tensor_copy(laP[j][:], la[j])
                    nc.vector.tensor_copy(la16[j][:], la[j])
                # init bf16 DK state
                for m_ in range(NROOT):
                    nc.vector.memset(
                        xrm16[:, m_ * WCOL:(m_ + 1) * WCOL], float(_DK_INIT[m_].real)
                    )
                    nc.vector.memset(
                        xim16[:, m_ * WCOL:(m_ + 1) * WCOL], float(_DK_INIT[m_].imag)
                    )
                nc.scalar.copy(xrm16[:, WBIG:], xrm16[:, 0:WMIR - WBIG])
                nc.scalar.copy(xim16[:, WBIG:], xim16[:, 0:WMIR - WBIG])

            # ============ stage B: Durand-Kerner, bf16 then fp32 ============
            def cmul_ops(dr, di, ar, ai, br, bi, ta, tb, eng=None):
                """The 6 TT ops of a complex multiply, as closures."""
                TT = (eng or nc.vector).tensor_tensor
                return [
                    lambda: TT(ta[:], ar[:], br[:], Alu.mult),
                    lambda: TT(tb[:], ai[:], bi[:], Alu.mult),
                    lambda: TT(dr[:], ta[:], tb[:], Alu.subtract),
                    lambda: TT(ta[:], ar[:], bi[:], Alu.mult),
                    lambda: TT(tb[:], ai[:], br[:], Alu.mult),
                    lambda: TT(di[:], ta[:], tb[:], Alu.add),
                ]

            def rr(*streams):
                """Round-robin the op streams into the emission order."""
                streams = [list(s) for s in streams]
                while any(streams):
                    for s in streams:
                        if s:
                            s.pop(0)()

            def dk_phase(tp, DT, n_iters, XRM, XIM, CB, pfx, last_keep=None):
                """Emit one DK phase on tiles of dtype DT.

                XRM/XIM: [128, WMIR] mirror-extended state, CB: coeff tiles.
                """
                xr = XRM[:, 0:WBIG]
                xi = XIM[:, 0:WBIG]

                def big(tag, dtype=DT, w=WBIG):
                    tag = pfx + tag
                    return tp.tile([PART, w], dtype, tag=tag, name=tag)

                EW = [None] + [WBIG + (WBIG - s * WCOL) for s in range(1, 5)] + [WBIG]
                er = [None] + [big(f"er{s}", w=EW[s]) for s in range(1, 6)]
                ei = [None] + [big(f"ei{s}", w=EW[s]) for s in range(1, 6)]

                t1 = big("t1")
                t2 = big("t2")
                yra, yrb = big("yra"), big("yrb")
                yia, yib = big("yia"), big("yib")
                p1r, p1i = big("p1r"), big("p1i")
                p2r, p2i = big("p2r"), big("p2i")
                gr = [big(f"g{s}r") for s in range(4)]
                gi = [big(f"g{s}i") for s in range(4)]
                dnr, dni = big("dnr"), big("dni")
                sq1, sq2 = big("sq1"), big("sq2")
                mag, rmag = big("mag"), big("rmag")
                qq = big("qq", w=2 * WBIG)     # adjacent qr|qi -> one clamp op
                qr_v, qi_v = qq[:, 0:WBIG], qq[:, WBIG:]

                pa, pb, pa2, pb2 = big("pa"), big("pb"), big("pa2"), big("pb2")
                gta = [big(f"gta{s}") for s in range(4)]
                gtb = [big(f"gtb{s}") for s in range(4)]

                # stride-0 broadcast views of the per-order coefficient
                # tiles: [128,63] -> [128,10,63] (block-repeated read)
                def r3(ap):
                    return ap.rearrange("p (c w) -> p c w", w=WCOL)

                CBB = [c[:].unsqueeze(1).broadcast_to([PART, NROOT, WCOL])
                       for c in CB]

                for it in range(n_iters):
                    # ---- stream A: polyval (Horner, real coeffs, c0 = 1)
                    A = [lambda: vadd(r3(yra[:]), r3(xr), CBB[0])]
                    yrs = [yra, yrb]
                    yis = [yia, yib]
                    for j in range(1, P):
                        yc, yn = yrs[(j - 1) % 2], yrs[j % 2]
                        yic, yin_ = yis[(j - 1) % 2], yis[j % 2]
                        yi_in = xi if j == 1 else yic[:]
                        A += [
                            (lambda yc=yc: vmul(pa[:], yc[:], xr)),
                            (lambda yi_in=yi_in: vmul(pb[:], yi_in, xi)),
                            (lambda yc=yc: vmul(pa2[:], yc[:], xi)),
                            (lambda yi_in=yi_in: vmul(pb2[:], yi_in, xr)),
                            (lambda: vsub(pa[:], pa[:], pb[:])),
                            (lambda yin_=yin_: vadd(yin_[:], pa2[:], pb2[:])),
                            (lambda yn=yn, j=j: vadd(r3(yn[:]), r3(pa[:]), CBB[j])),
                        ]
                    pr_, pi_ = yrs[(P - 1) % 2], yis[(P - 1) % 2]

                    # ---- stream B: diffs + ACT extends + denominator product
                    Bs = []
                    for s in range(1, 6):
                        Bs.append(lambda s=s: vrsub(
                            er[s][:, 0:WBIG], XRM[:, s * WCOL:s * WCOL + WBIG], xr))
                        Bs.append(lambda s=s: vrsub(
                            ei[s][:, 0:WBIG], XIM[:, s * WCOL:s * WCOL + WBIG], xi))
                        if s < 5:
                            ext = EW[s] - WBIG
                            Bs.append(lambda s=s, ext=ext: nc.scalar.copy(
                                er[s][:, WBIG:], er[s][:, 0:ext]))
                            Bs.append(lambda s=s, ext=ext: nc.scalar.copy(
                                ei[s][:, WBIG:], ei[s][:, 0:ext]))
                    # 4 g-cmuls, 4-way round-robin (dep distance 4)
                    glists = []
                    for s in range(1, 5):
                        off = (NROOT - s) * WCOL
                        glists.append(cmul_ops(
                            gr[s - 1], gi[s - 1],
                            er[s][:, 0:WBIG], ei[s][:, 0:WBIG],
                            er[s][:, off:off + WBIG], ei[s][:, off:off + WBIG],
                            gta[s - 1], gtb[s - 1]))
                    for k in range(6):
                        for gl in glists:
                            Bs.append(gl[k])
                    # p1 = g1*g2, p2 = g3*g4 (2-way interleave)
                    c1 = cmul_ops(p1r, p1i, gr[0], gi[0], gr[1], gi[1], gta[0], gtb[0])
                    c2 = cmul_ops(p2r, p2i, gr[2], gi[2], gr[3], gi[3], gta[1], gtb[1])
                    for k in range(6):
                        Bs.append(c1[k])
                        Bs.append(c2[k])
                    # dnr = p1*p2; then D = dnr*e5 (writes p1r/p1i, free again)
                    Bs += cmul_ops(dnr, dni, p1r, p1i, p2r, p2i, gta[2], gtb[2])
                    Bs += cmul_ops(p1r, p1i, dnr, dni,
                                   er[5][:, 0:WBIG], ei[5][:, 0:WBIG],
                                   gta[3], gtb[3])

                    rr(A, Bs)
                    drp, dip = p1r, p1i

                    # ---- q = p * conj(D) / (|D|^2 + 1e-30)
                    nc.scalar.activation(sq1[:], drp[:], Act.Square)
                    nc.scalar.activation(sq2[:], dip[:], Act.Square)
                    vadd(mag[:], sq1[:], sq2[:])
                    nc.vector.tensor_scalar(mag[:], mag[:], 1e-30, None, Alu.add)
                    nc.vector.reciprocal(rmag[:], mag[:])
                    vmul(t1[:], pr_[:], drp[:])
                    vmul(t2[:], pi_[:], dip[:])
                    vmul(pa[:], pi_[:], drp[:])
                    vmul(pb[:], pr_[:], dip[:])
                    vadd(qr_v, t1[:], t2[:])
                    vsub(qi_v, pa[:], pb[:])
                    vmul(qr_v, qr_v, rmag[:])
                    vmul(qi_v, qi_v, rmag[:])
                    # single fused clamp over the adjacent qr|qi halves
                    nc.vector.tensor_scalar(qq[:], qq[:], Q_CLAMP, -Q_CLAMP,
                                            Alu.min, Alu.max)
                    last = (it == n_iters - 1)
                    vsub(xr, xr, qr_v)
                    vsub(xi, xi, qi_v)
                    if not (last and last_keep == "skip_mirror"):
                        nc.vector.tensor_copy(XRM[:, WBIG:], XRM[:, 0:WMIR - WBIG])
                        nc.vector.tensor_copy(XIM[:, WBIG:], XIM[:, 0:WMIR - WBIG])

            with tc.tile_pool(name="dk16", bufs=1) as tp16:
                dk_phase(tp16, BF16, N_IT_BF, xrm16, xim16, la16, "h_",
                         last_keep="skip_mirror")

            # phase boundary: widen bf16 state to fp32 (full mirror range
            # is stale on the last bf16 iter; re-extend after conversion)
            nc.vector.tensor_copy(xrm[:, 0:WBIG], xrm16[:, 0:WBIG])
            nc.vector.tensor_copy(xim[:, 0:WBIG], xim16[:, 0:WBIG])
            nc.scalar.copy(xrm[:, WBIG:], xrm[:, 0:WMIR - WBIG])
            nc.scalar.copy(xim[:, WBIG:], xim[:, 0:WMIR - WBIG])

            with tc.tile_pool(name="dk32", bufs=1) as tp32:
                dk_phase(tp32, F32, N_IT_F32, xrm, xim, laP, "f_",
                         last_keep="skip_mirror")

            # ---- formants: angle, validity, partial sort, normalize ----
            with tc.tile_pool(name="post", bufs=1) as tp:
                xr = xrm[:, 0:WBIG]
                xi = xim[:, 0:WBIG]

                def big(tag, dtype=F32, w=WBIG):
                    return tp.tile([PART, w], dtype, tag=tag, name=tag)

                rx = big("po_rx")
                nc.vector.reciprocal(rx[:], xr)
                tt_ = big("po_t")
                vmul(tt_[:], xi, rx[:])
                nc.vector.tensor_scalar(tt_[:], tt_[:], 1e20, -1e20, Alu.min, Alu.max)
                ang = big("po_ang")
                nc.scalar.activation(ang[:], tt_[:], Act.Arctan)
                neg = big("po_neg", dtype=U8)
                nc.vector.tensor_scalar(neg[:], xr, 0.0, None, Alu.is_lt)
                shifted = big("po_shift")
                nc.vector.tensor_scalar(shifted[:], ang[:], float(PI), None, Alu.add)
                nc.vector.copy_predicated(ang[:], neg[:], shifted[:])

                m1 = big("po_m1", dtype=U8)
                nc.vector.tensor_scalar(m1[:], xi, 0.0, None, Alu.is_gt)
                m2 = big("po_m2", dtype=U8)
                nc.vector.tensor_scalar(m2[:], ang[:], float(ANG_LO), None, Alu.is_gt)
                m3 = big("po_m3", dtype=U8)
                nc.vector.tensor_scalar(m3[:], ang[:], float(ANG_HI), None, Alu.is_lt)
                stt(m1[:], m1[:], m2[:], Alu.logical_and)
                stt(m1[:], m1[:], m3[:], Alu.logical_and)
                angv = big("po_angv")
                nc.vector.memset(angv[:], float(ANG_INVALID))
                nc.vector.copy_predicated(angv[:], m1[:], ang[:])

                # partial selection sort (4 bubble passes over 10 blocks)
                cur = [angv[:, m_ * WCOL:(m_ + 1) * WCOL] for m_ in range(NROOT)]
                for k_ in range(4):
                    for i in range(NROOT - 1, k_, -1):
                        lo = tp.tile([PART, WCOL], F32, tag=f"srt{k_}_{i}a", name=f"srt{k_}_{i}a")
                        hi = tp.tile([PART, WCOL], F32, tag=f"srt{k_}_{i}b", name=f"srt{k_}_{i}b")
                        stt(lo[:], cur[i - 1], cur[i], Alu.min)
                        stt(hi[:], cur[i - 1], cur[i], Alu.max)
                        cur[i - 1] = lo[:]
                        cur[i] = hi[:]
                ot = tp.tile([PART, 4 * WCOL], F32, tag="srt_out", name="srt_out")
                for k_ in range(4):
                    nc.vector.tensor_scalar(
                        ot[:, k_ * WCOL:(k_ + 1) * WCOL], cur[k_],
                        float(OUT_SCALE), -1.0, Alu.mult, Alu.add
                    )
                nc.sync.dma_start(out=out_d[:], in_=ot[:])

    lowp.__exit__(None, None, None)
    _split_multi_waits(nc)
    return nc


# ------------------------------------------------------------- jit runner
_CACHE = {}


def _get_runner():
    if "runner" in _CACHE:
        return _CACHE["runner"]

    import jax
    from jax.sharding import Mesh, PartitionSpec, NamedSharding
    from jax.experimental.shard_map import shard_map
    from concourse import bass2jax
    from concourse import mybir

    nc = _build_module()
    bass2jax.install_neuronx_cc_hook()

    partition_name = nc.partition_id_tensor.name if nc.partition_id_tensor else None
    in_names, out_names, out_avals = [], [], []
    for alloc in nc.m.functions[0].allocations:
        if not isinstance(alloc, mybir.MemoryLocationSet):
            continue
        name = alloc.memorylocations[0].name
        if alloc.kind == "ExternalInput":
            if name != partition_name:
                in_names.append(name)
        elif alloc.kind == "ExternalOutput":
            shape = tuple(alloc.tensor_shape)
            out_names.append(name)
            out_avals.append(jax.core.ShapedArray(shape, mybir.dt.np(alloc.dtype)))
    all_in = in_names + out_names + ([partition_name] if partition_name else [])

    def _body(*args):
        operands = list(args)
        if partition_name:
            operands.append(bass2jax.partition_id_tensor())
        return tuple(bass2jax._bass_exec_p.bind(
            *operands, out_avals=tuple(out_avals), in_names=tuple(all_in),
            out_names=tuple(out_names), lowering_input_output_aliases=(),
            sim_require_finite=True, sim_require_nnan=True, nc=nc))

    mesh = Mesh(np.asarray(jax.devices()[:NCORES]), ("core",))
    n_args = len(in_names) + len(out_names)
    sharded = jax.jit(
        shard_map(_body, mesh=mesh, in_specs=(PartitionSpec("core"),) * n_args,
                  out_specs=(PartitionSpec("core"),) * len(out_names), check_rep=False),
        keep_unused=True)
    sh = NamedSharding(mesh, PartitionSpec("core"))

    # constants + zero output placeholders: device-resident once
    cs, wm = _spec_consts()
    const_dev = {
        "cs": jax.device_put(np.tile(cs, (NCORES, 1)), sh),
        "wm": jax.device_put(np.tile(wm, (NCORES, 1)), sh),
    }
    zeros_dev = [
        jax.device_put(np.zeros((NCORES * a.shape[0], *a.shape[1:]),
                                np.dtype(a.dtype)), sh)
        for a in out_avals
    ]
    jax.block_until_ready(list(const_dev.values()) + zeros_dev)

    runner = {
        "jax": jax, "sharded": sharded, "sh": sh,
        "in_names": in_names, "out_names": out_names, "out_avals": out_avals,
        "const_dev": const_dev, "zeros_dev": zeros_dev,
    }
    _CACHE["runner"] = runner
    return runner


def _prep_kin(r_coeff):
    r_coeff = np.ascontiguousarray(r_coeff, dtype=np.float32)
    kin = np.zeros((NCORES, P, FPAD), np.float32)
    for c in range(NCORES):
        chunk = r_coeff[c * BPC:(c + 1) * BPC]               # (8, 10, 1000)
        kin[c, :, :FPC] = np.transpose(chunk, (1, 0, 2)).reshape(P, FPC)
    # (core, c, p, w) -> (core, p, (c,w)): per-partition contiguous rows
    kin = kin.reshape(NCORES, P, PART, WCOL).transpose(0, 2, 1, 3)
    return np.ascontiguousarray(kin).reshape(NCORES * PART, P * WCOL)


def kernel(r_coeff: np.ndarray) -> np.ndarray:
    import time as _time

    r = _get_runner()
    jax = r["jax"]
    kin = _prep_kin(r_coeff)

    t0 = _time.time()
    args = []
    for n in r["in_names"]:
        args.append(kin if n == "kin" else r["const_dev"][n])
    outs = r["sharded"](*args, *r["zeros_dev"])
    res = np.asarray(outs[0])
    _CACHE["exec_wall_s"] = _time.time() - t0
    _CACHE["last_args"] = args

    res = res.reshape(NCORES, PART, 4, WCOL).transpose(0, 2, 1, 3)
    out = np.empty((B, 4, T), np.float32)
    for c in range(NCORES):
        o = res[c].reshape(4, FPAD)[:, :FPC]
        out[c * BPC:(c + 1) * BPC] = np.transpose(o.reshape(4, BPC, T), (1, 0, 2))
    return out


def _amortized_exec_ns(n=33, trials=3):
    """Estimate on-device exec time per call: dispatch n async calls in one
    flush vs 1; the difference divided by n-1 removes the RTT floor."""
    import time as _time

    r = _get_runner()
    jax = r["jax"]
    args = _CACHE["last_args"]
    dev_args = [a if hasattr(a, "sharding") else jax.device_put(a, r["sh"])
                for a in args]
    jax.block_until_ready(dev_args)

    def flush(k):
        t0 = _time.time()
        rs = [r["sharded"](*dev_args, *r["zeros_dev"]) for _ in range(k)]
        jax.block_until_ready(rs)
        return _time.time() - t0

    flush(1)
    best = None
    for _ in range(trials):
        t1 = min(flush(1) for _ in range(3))
        tN = min(flush(n) for _ in range(2))
        est = (tN - t1) / (n - 1)
        best = est if best is None else min(best, est)
    return int(best * 1e9)


# revision 26
# speedup vs baseline: 1.0318x; 1.0318x over previous
"""Trainium2 Bass kernel for the CycleConsistency formant-extraction pipeline.

Pipeline per frame (64*1000 = 64000 independent frames):
  reflection coeffs (10) -> step-up recursion -> predictor poly A (11)
  -> power spectrum at 116 rfft bins -> autocorrelation (11 lags)
  -> Levinson-Durbin -> monic allpole poly (11)
  -> Durand-Kerner (clamped) -> 10 roots
  -> angles -> validity mask -> partial sort -> lowest 4 -> normalize

Sharding: batch dim across 8 cores (8 batches/core = 8000 frames/core,
padded to 8064 = 128*63).  SoA layout: per-frame scalar = [128, 63] tile
(frame = partition*63 + col); Durand-Kerner state = [128, 630]
(10 root-major blocks of 63).

v8: mixed-precision Durand-Kerner + DMA layout fixes.
  HW-measured per-[128,630]-op DVE costs: tensor_tensor fp32 ~577ns,
  bf16 ~363ns (2x_1p), scalar_tensor_tensor ~629ns regardless of dtype
  (no DVE perf mode), tensor_scalar imm fp32 ~381 / bf16 ~180 (2x_2p /
  4x_2p).  So the DK loop is emitted as plain tensor_tensor ops, runs
  14 iterations in bf16 and 4 in fp32 (HW rel err 1.34e-2 vs the 2e-2
  gate).  The denominator eps guard is a ts-add of 1e-30 before the
  reciprocal, keeping every op a plain TT/TS with no NaN paths.
  Coefficients are read through stride-0 broadcast APs (no block
  broadcast copies).  kin/out DRAM layouts are per-partition contiguous
  and the two transposing stage-A DMAs are split across the three
  DMA-capable queues (SP/ACT/Pool) and pipelined chunk-wise with the
  spectrum matmuls.
"""

import numpy as np

# ---------------------------------------------------------------- constants
B, P, T = 64, 10, 1000
NCORES = 8
BPC = B // NCORES            # batches per core
FPC = BPC * T                # frames per core (8000)
PART = 128
WCOL = 63                    # columns per SoA tile
FPAD = PART * WCOL           # padded frames per core (8064)
NROOT = P
WBIG = WCOL * NROOT          # 630
WMIR = WBIG + 9 * WCOL       # 1197: mirror-extended x tiles
NF = 116                     # spectrum bins
NCHUNK = 16
CH = FPAD // NCHUNK          # 504 (one psum bank)
N_IT_BF = 15                 # bf16 DK iterations
N_IT_F32 = 4                 # fp32 DK iterations
Q_CLAMP = 2.0
RMAG_CLAMP = 1e30

FM_SR = 10000.0
RC_SR = 22050.0
ANG_LO = np.float32(50.0 * 2.0 * np.pi / FM_SR)
ANG_HI = np.float32((FM_SR / 2 - 50.0) * 2.0 * np.pi / FM_SR)
ANG_INVALID = np.float32(2.0 * np.pi)
OUT_SCALE = np.float32((FM_SR / (2.0 * np.pi)) * 2.0 / (RC_SR / 2.0))
PI = np.float32(np.pi)

_DK_INIT = ((0.4 + 0.9j) ** np.arange(1, P + 1)).astype(np.complex64)


def _spec_consts():
    j = np.arange(P + 1)[:, None]
    k = np.arange(NF)[None, :]
    C = np.cos(2 * np.pi * j * k / 512.0).astype(np.float32)   # [11, 116]
    S = np.sin(2 * np.pi * j * k / 512.0).astype(np.float32)   # [11, 116]
    kk = np.arange(NF)[:, None]
    m = np.arange(P + 1)[None, :]
    cc = np.full((NF, 1), 2.0)
    cc[0] = 1.0
    cc[NF - 1] = 1.0
    W = ((1.0 / 230.0) * cc * np.cos(2 * np.pi * kk * m / 230.0)).astype(np.float32)
    return np.concatenate([C, S], axis=1), W   # [11, 232], [116, 11]


# ------------------------------------------------------- tile workarounds
def _install_tile_patches():
    import bass_rust
    import concourse.tile as tile
    from concourse.vector_clock import ScopedClock

    if getattr(tile.TileContext, "_drain_patched", False):
        return

    def _drain_and_barrier(self, tick_clock, wait_clock):
        # this walrus build accepts only ONE sync-wait command per
        # instruction; fan the tail-drain waits out over NOPs.
        gc = tick_clock.global_clock
        n = len(gc)
        for i in [i for i in range(n) if gc[i] > 0]:
            partial = bass_rust.VectorClock(
                [gc[j] if j == i else 0 for j in range(n)]
            )
            nop = self.nc.sync.nop()
            wait_clock.add_sem_waits(nop.ins, ScopedClock({None: partial}))
        self.nc.sync.drain()
        self.nc.all_engine_barrier()
        popped = self.nc._tile_sem_poison_stack.pop()
        assert popped is self._sem_poison
        self.nc.clear_and_free_semaphores(list(self.sems.allocated().values()))
        self.nc.all_engine_barrier()

    tile.TileContext._drain_and_barrier = _drain_and_barrier
    tile.TileContext._drain_patched = True


def _split_multi_waits(nc):
    import concourse.mybir as mybir

    ctr = 0
    for func in nc.m.functions:
        for bb in func.blocks:
            out = []
            for ins in bb.instructions:
                si = ins.sync_info
                if si is not None and si.on_wait is not None and len(si.on_wait) > 1:
                    waits = list(si.on_wait)
                    for w in waits[:-1]:
                        nop = mybir.InstNoOp(name=f"I-ws{ctr}")
                        ctr += 1
                        nop.engine = ins.engine
                        nop.sync_info = mybir.SyncInfo(on_wait=[w], on_update=[])
                        out.append(nop)
                    ins.sync_info = mybir.SyncInfo(
                        on_wait=[waits[-1]],
                        on_update=list(si.on_update) if si.on_update else [],
                    )
                out.append(ins)
            bb.instructions[:] = out


# ------------------------------------------------------------- bass module
def _build_module():
    import concourse.bass as bass
    import concourse.mybir as mybir
    import concourse.tile as tile

    _install_tile_patches()

    F32 = mybir.dt.float32
    BF16 = mybir.dt.bfloat16
    U8 = mybir.dt.uint8
    Alu = mybir.AluOpType
    Act = mybir.ActivationFunctionType

    nc = bass.Bass()
    # kin rows are per-partition contiguous (p, (c,w)) so the load is one
    # DMA with 2520B descriptors instead of 1280 x 252B
    kin = nc.dram_tensor("kin", [PART, P * WCOL], F32, kind="ExternalInput")
    cs_d = nc.dram_tensor("cs", [P + 1, 2 * NF], F32, kind="ExternalInput")
    wm_d = nc.dram_tensor("wm", [NF, P + 1], F32, kind="ExternalInput")
    out_d = nc.dram_tensor("out", [PART, 4 * WCOL], F32, kind="ExternalOutput")
    alay_d = nc.dram_tensor("alay", [P + 1, FPAD], F32)
    rlay_d = nc.dram_tensor("rlay", [P + 1, FPAD], F32)

    lowp = nc.allow_low_precision(reason="bf16 DK phase; validated vs 2e-2 gate")
    lowp.__enter__()

    with tile.TileContext(nc) as tc:
        with tc.tile_pool(name="persist", bufs=1) as pp:

            # STT helpers (stage A, [128,63] tiles where fusion still wins).
            def stt(out, a, b, op1, s=1.0, op0=Alu.mult, eng=None):
                (eng or nc.vector).scalar_tensor_tensor(out, a, float(s), b, op0, op1)

            # plain tensor_tensor emitters (stage B workhorses)
            def vmul(out, a, b):      # out = a*b
                nc.vector.tensor_tensor(out, a, b, Alu.mult)

            def vadd(out, a, b):      # out = a+b
                nc.vector.tensor_tensor(out, a, b, Alu.add)

            def vsub(out, a, b):      # out = a-b
                nc.vector.tensor_tensor(out, a, b, Alu.subtract)

            def vrsub(out, a, b):     # out = b-a
                nc.vector.tensor_tensor(out, b, a, Alu.subtract)

            # persistent LPC coefficients: one [128,63] tile per order;
            # the DK loop reads them through stride-0 broadcast APs
            laP = [pp.tile([PART, WCOL], F32, tag=f"lap{j}", name=f"lap{j}")
                   for j in range(P)]
            la16 = [pp.tile([PART, WCOL], BF16, tag=f"la16_{j}", name=f"la16_{j}")
                    for j in range(P)]
            # DK state: bf16 phase-1 tiles and fp32 phase-2 tiles
            xrm16 = pp.tile([PART, WMIR], BF16, tag="xrm16", name="xrm16")
            xim16 = pp.tile([PART, WMIR], BF16, tag="xim16", name="xim16")
            xrm = pp.tile([PART, WMIR], F32, tag="xrm", name="xrm")
            xim = pp.tile([PART, WMIR], F32, tag="xim", name="xim")

            # ============ stage A: everything before Durand-Kerner ============
            with tc.tile_pool(name="pre", bufs=1) as prep, \
                 tc.tile_pool(name="pret", bufs=2) as pret, \
                 tc.tile_pool(name="psum", bufs=2, space="PSUM") as psp:

                # ---- load K (single contiguous DMA), forward levinson ----
                ka = prep.tile([PART, P * WCOL], F32, tag="ka", name="ka")
                nc.sync.dma_start(out=ka[:], in_=kin[:])
                kt = [ka[:, p_ * WCOL:(p_ + 1) * WCOL] for p_ in range(P)]

                a_all = prep.tile([PART, P * WCOL], F32, tag="a_all", name="a_all")
                a = [kt[0]]            # list of APs
                for p_ in range(1, P):
                    kp = kt[p_]
                    na = []
                    for i in range(p_):
                        prod = pret.tile([PART, WCOL], F32, tag="fl_prod", name="fl_prod")
                        vmul(prod[:], kp, a[p_ - 1 - i])
                        if p_ == P - 1:
                            s = a_all[:, i * WCOL:(i + 1) * WCOL]
                        else:
                            s = prep.tile([PART, WCOL], F32, tag=f"a{p_}_{i}", name=f"a{p_}_{i}")[:]
                        vadd(s, a[i], prod[:])
                        na.append(s)
                    na.append(kp)
                    a = na
                nc.scalar.copy(a_all[:, (P - 1) * WCOL:], kt[P - 1])

                # transposing DMA a_all -> alay rows 1..10 has 252B
                # descriptors; split across 4 DGE queues by partition range
                qsplit = [(nc.sync, slice(0, 43)), (nc.scalar, slice(43, 86)),
                          (nc.gpsimd, slice(86, PART))]
                for qeng, pr in qsplit:
                    qeng.dma_start(
                        out=alay_d[1:].rearrange("c (p w) -> p c w",
                                                 p=PART)[pr],
                        in_=a_all[pr].rearrange("p (c w) -> p c w", w=WCOL),
                    )
                A_lay = prep.tile([P + 1, FPAD], F32, tag="A_lay", name="A_lay")
                nc.vector.memset(A_lay[0:1, :], 1.0)
                for ch in range(NCHUNK):
                    sl = slice(ch * CH, (ch + 1) * CH)
                    nc.sync.dma_start(out=A_lay[1:, sl], in_=alay_d[1:, sl])

                # ---- spectrum + autocorrelation (TensorE matmuls) ----
                cs = prep.tile([P + 1, 2 * NF], F32, tag="cs", name="cs")
                nc.sync.dma_start(out=cs[:], in_=cs_d[:])
                wm = prep.tile([NF, P + 1], F32, tag="wm", name="wm")
                nc.sync.dma_start(out=wm[:], in_=wm_d[:])
                r_lay = prep.tile([P + 1, FPAD], F32, tag="r_lay", name="r_lay")

                for ch in range(NCHUNK):
                    sl = slice(ch * CH, (ch + 1) * CH)
                    ps_re = psp.tile([NF, CH], F32, tag="ps_re", name="ps_re")
                    ps_im = psp.tile([NF, CH], F32, tag="ps_im", name="ps_im")
                    nc.tensor.matmul(ps_re[:], cs[:, 0:NF], A_lay[:, sl], start=True, stop=True)
                    nc.tensor.matmul(ps_im[:], cs[:, NF:2 * NF], A_lay[:, sl], start=True, stop=True)
                    sq_re = pret.tile([NF, CH], F32, tag="sq_re", name="sq_re")
                    sq_im = pret.tile([NF, CH], F32, tag="sq_im", name="sq_im")
                    nc.scalar.activation(sq_re[:], ps_re[:], Act.Square)
                    nc.scalar.activation(sq_im[:], ps_im[:], Act.Square)
                    spec = pret.tile([NF, CH], F32, tag="spec", name="spec")
                    vadd(spec[:], sq_re[:], sq_im[:])
                    ps_r = psp.tile([P + 1, CH], F32, tag="ps_r", name="ps_r")
                    nc.tensor.matmul(ps_r[:], wm[:], spec[:], start=True, stop=True)
                    nc.vector.tensor_copy(r_lay[:, sl], ps_r[:])

                r_all = prep.tile([PART, (P + 1) * WCOL], F32, tag="r_all", name="r_all")
                qcycle = [nc.sync, nc.scalar, nc.gpsimd]
                for ch in range(NCHUNK):
                    sl = slice(ch * CH, (ch + 1) * CH)
                    nc.sync.dma_start(out=rlay_d[:, sl], in_=r_lay[:, sl])
                    band = slice(ch * (PART // NCHUNK), (ch + 1) * (PART // NCHUNK))
                    qcycle[ch % 3].dma_start(
                        out=r_all[band].rearrange("p (c w) -> p c w", w=WCOL),
                        in_=rlay_d.rearrange("c (p w) -> p c w", p=PART)[band],
                    )
                r = [r_all[:, m_ * WCOL:(m_ + 1) * WCOL] for m_ in range(P + 1)]

                # ---- Levinson-Durbin (SoA; everything passed as APs) ----
                def div_newton(num, den, tag, negate=False):
                    """q = (-)num/den via DVE reciprocal (shorter serial
                    chain; DVE recip is accurate enough for the LD k's)."""
                    rc_ = pret.tile([PART, WCOL], F32, tag="ldv_rc", name="ldv_rc")
                    nc.vector.reciprocal(rc_[:], den)
                    q = prep.tile([PART, WCOL], F32, tag=tag, name=tag)
                    if negate:
                        stt(q[:], num, rc_[:], Alu.mult, s=-1.0)
                    else:
                        vmul(q[:], num, rc_[:])
                    return q[:]

                k0 = div_newton(r[1], r[0], "ld_k0", negate=True)
                la = [k0]
                err = prep.tile([PART, WCOL], F32, tag="ld_err", name="ld_err")
                om = pret.tile([PART, WCOL], F32, tag="ld_om", name="ld_om")
                ksq = pret.tile([PART, WCOL], F32, tag="ld_ksq", name="ld_ksq")
                stt(ksq[:], k0, k0, Alu.mult, s=-1.0)   # -k0^2
                nc.vector.tensor_scalar(om[:], ksq[:], 1.0, None, Alu.add)
                vmul(err[:], r[0], om[:])
                for m_ in range(1, P):
                    acc = pret.tile([PART, WCOL], F32, tag="ld_acc", name="ld_acc")
                    nc.vector.tensor_copy(acc[:], r[m_ + 1])
                    for i in range(m_):
                        prd = pret.tile([PART, WCOL], F32, tag="ld_p", name="ld_p")
                        vmul(prd[:], la[i], r[m_ - i])
                        vadd(acc[:], acc[:], prd[:])
                    kk = div_newton(acc[:], err[:], f"ld_k{m_}", negate=True)
                    nla = []
                    for i in range(m_):
                        prd = pret.tile([PART, WCOL], F32, tag="ld_p2", name="ld_p2")
                        vmul(prd[:], kk, la[m_ - 1 - i])
                        s = prep.tile([PART, WCOL], F32, tag=f"c{m_}_{i}", name=f"c{m_}_{i}")
                        vadd(s[:], la[i], prd[:])
                        nla.append(s[:])
                    nla.append(kk)
                    la = nla
                    if m_ < P - 1:
                        ksq2 = pret.tile([PART, WCOL], F32, tag="ld_ksq2", name="ld_ksq2")
                        stt(ksq2[:], kk, kk, Alu.mult, s=-1.0)   # -k^2
                        om2 = pret.tile([PART, WCOL], F32, tag="ld_om2", name="ld_om2")
                        nc.vector.tensor_scalar(om2[:], ksq2[:], 1.0, None, Alu.add)
                        vmul(err[:], err[:], om2[:])

                # persist coefficients (fp32 + bf16 conversion copies)
                for j in range(P):
                    nc.vector.tensor_copy(laP[j][:], la[j])
                    nc.vector.tensor_copy(la16[j][:], la[j])
                # init bf16 DK state
                for m_ in range(NROOT):
                    nc.vector.memset(
                        xrm16[:, m_ * WCOL:(m_ + 1) * WCOL], float(_DK_INIT[m_].real)
                    )
                    nc.vector.memset(
                        xim16[:, m_ * WCOL:(m_ + 1) * WCOL], float(_DK_INIT[m_].imag)
                    )
                nc.scalar.copy(xrm16[:, WBIG:], xrm16[:, 0:WMIR - WBIG])
                nc.scalar.copy(xim16[:, WBIG:], xim16[:, 0:WMIR - WBIG])

            # ============ stage B: Durand-Kerner, bf16 then fp32 ============
            def cmul_ops(dr, di, ar, ai, br, bi, ta, tb, eng=None):
                """The 6 TT ops of a complex multiply, as closures."""
                TT = (eng or nc.vector).tensor_tensor
                return [
                    lambda: TT(ta[:], ar[:], br[:], Alu.mult),
                    lambda: TT(tb[:], ai[:], bi[:], Alu.mult),
                    lambda: TT(dr[:], ta[:], tb[:], Alu.subtract),
                    lambda: TT(ta[:], ar[:], bi[:], Alu.mult),
                    lambda: TT(tb[:], ai[:], br[:], Alu.mult),
                    lambda: TT(di[:], ta[:], tb[:], Alu.add),
                ]

            def rr(*streams):
                """Round-robin the op streams into the emission order."""
                streams = [list(s) for s in streams]
                while any(streams):
                    for s in streams:
                        if s:
                            s.pop(0)()

            def dk_phase(tp, DT, n_iters, XRM, XIM, CB, pfx, last_keep=None):
                """Emit one DK phase on tiles of dtype DT.

                XRM/XIM: [128, WMIR] mirror-extended state, CB: coeff tiles.
                """
                xr = XRM[:, 0:WBIG]
                xi = XIM[:, 0:WBIG]

                def big(tag, dtype=DT, w=WBIG):
                    tag = pfx + tag
                    return tp.tile([PART, w], dtype, tag=tag, name=tag)

                EW = [None] + [WBIG + (WBIG - s * WCOL) for s in range(1, 5)] + [WBIG]
                er = [None] + [big(f"er{s}", w=EW[s]) for s in range(1, 6)]
                ei = [None] + [big(f"ei{s}", w=EW[s]) for s in range(1, 6)]

                t1 = big("t1")
                t2 = big("t2")
                yra, yrb = big("yra"), big("yrb")
                yia, yib = big("yia"), big("yib")
                p1r, p1i = big("p1r"), big("p1i")
                p2r, p2i = big("p2r"), big("p2i")
                gr = [big(f"g{s}r") for s in range(4)]
                gi = [big(f"g{s}i") for s in range(4)]
                dnr, dni = big("dnr"), big("dni")
                sq1, sq2 = big("sq1"), big("sq2")
                mag, rmag = big("mag"), big("rmag")
                qq = big("qq", w=2 * WBIG)     # adjacent qr|qi -> one clamp op
                qr_v, qi_v = qq[:, 0:WBIG], qq[:, WBIG:]

                pa, pb, pa2, pb2 = big("pa"), big("pb"), big("pa2"), big("pb2")
                gta = [big(f"gta{s}") for s in range(4)]
                gtb = [big(f"gtb{s}") for s in range(4)]

                # stride-0 broadcast views of the per-order coefficient
                # tiles: [128,63] -> [128,10,63] (block-repeated read)
                def r3(ap):
                    return ap.rearrange("p (c w) -> p c w", w=WCOL)

                CBB = [c[:].unsqueeze(1).broadcast_to([PART, NROOT, WCOL])
                       for c in CB]

                for it in range(n_iters):
                    # ---- stream A: polyval (Horner, real coeffs, c0 = 1)
                    A = [lambda: vadd(r3(yra[:]), r3(xr), CBB[0])]
                    yrs = [yra, yrb]
                    yis = [yia, yib]
                    for j in range(1, P):
                        yc, yn = yrs[(j - 1) % 2], yrs[j % 2]
                        yic, yin_ = yis[(j - 1) % 2], yis[j % 2]
                        yi_in = xi if j == 1 else yic[:]
                        A += [
                            (lambda yc=yc: vmul(pa[:], yc[:], xr)),
                            (lambda yi_in=yi_in: vmul(pb[:], yi_in, xi)),
                            (lambda yc=yc: vmul(pa2[:], yc[:], xi)),
                            (lambda yi_in=yi_in: vmul(pb2[:], yi_in, xr)),
                            (lambda: vsub(pa[:], pa[:], pb[:])),
                            (lambda yin_=yin_: vadd(yin_[:], pa2[:], pb2[:])),
                            (lambda yn=yn, j=j: vadd(r3(yn[:]), r3(pa[:]), CBB[j])),
                        ]
                    pr_, pi_ = yrs[(P - 1) % 2], yis[(P - 1) % 2]

                    # ---- stream B: diffs + ACT extends + denominator product
                    Bs = []
                    for s in range(1, 6):
                        Bs.append(lambda s=s: vrsub(
                            er[s][:, 0:WBIG], XRM[:, s * WCOL:s * WCOL + WBIG], xr))
                        Bs.append(lambda s=s: vrsub(
                            ei[s][:, 0:WBIG], XIM[:, s * WCOL:s * WCOL + WBIG], xi))
                        if s < 5:
                            ext = EW[s] - WBIG
                            Bs.append(lambda s=s, ext=ext: nc.scalar.copy(
                                er[s][:, WBIG:], er[s][:, 0:ext]))
                            Bs.append(lambda s=s, ext=ext: nc.scalar.copy(
                                ei[s][:, WBIG:], ei[s][:, 0:ext]))
                    # 4 g-cmuls, 4-way round-robin (dep distance 4)
                    glists = []
                    for s in range(1, 5):
                        off = (NROOT - s) * WCOL
                        glists.append(cmul_ops(
                            gr[s - 1], gi[s - 1],
                            er[s][:, 0:WBIG], ei[s][:, 0:WBIG],
                            er[s][:, off:off + WBIG], ei[s][:, off:off + WBIG],
                            gta[s - 1], gtb[s - 1]))
                    for k in range(6):
                        for gl in glists:
                            Bs.append(gl[k])
                    # p1 = g1*g2, p2 = g3*g4 (2-way interleave)
                    c1 = cmul_ops(p1r, p1i, gr[0], gi[0], gr[1], gi[1], gta[0], gtb[0])
                    c2 = cmul_ops(p2r, p2i, gr[2], gi[2], gr[3], gi[3], gta[1], gtb[1])
                    for k in range(6):
                        Bs.append(c1[k])
                        Bs.append(c2[k])
                    # dnr = p1*p2; then D = dnr*e5 (writes p1r/p1i, free again)
                    Bs += cmul_ops(dnr, dni, p1r, p1i, p2r, p2i, gta[2], gtb[2])
                    Bs += cmul_ops(p1r, p1i, dnr, dni,
                                   er[5][:, 0:WBIG], ei[5][:, 0:WBIG],
                                   gta[3], gtb[3])

                    rr(A, Bs)
                    drp, dip = p1r, p1i

                    # ---- q = p * conj(D) / (|D|^2 + 1e-30)
                    nc.scalar.activation(sq1[:], drp[:], Act.Square)
                    nc.scalar.activation(sq2[:], dip[:], Act.Square)
                    vadd(mag[:], sq1[:], sq2[:])
                    nc.vector.tensor_scalar(mag[:], mag[:], 1e-30, None, Alu.add)
                    nc.vector.reciprocal(rmag[:], mag[:])
                    vmul(t1[:], pr_[:], drp[:])
                    vmul(t2[:], pi_[:], dip[:])
                    vmul(pa[:], pi_[:], drp[:])
                    vmul(pb[:], pr_[:], dip[:])
                    vadd(qr_v, t1[:], t2[:])
                    vsub(qi_v, pa[:], pb[:])
                    vmul(qr_v, qr_v, rmag[:])
                    vmul(qi_v, qi_v, rmag[:])
                    # single fused clamp over the adjacent qr|qi halves
                    nc.vector.tensor_scalar(qq[:], qq[:], Q_CLAMP, -Q_CLAMP,
                                            Alu.min, Alu.max)
                    last = (it == n_iters - 1)
                    vsub(xr, xr, qr_v)
                    vsub(xi, xi, qi_v)
                    if not (last and last_keep == "skip_mirror"):
                        nc.vector.tensor_copy(XRM[:, WBIG:], XRM[:, 0:WMIR - WBIG])
                        nc.vector.tensor_copy(XIM[:, WBIG:], XIM[:, 0:WMIR - WBIG])

            with tc.tile_pool(name="dk16", bufs=1) as tp16:
                dk_phase(tp16, BF16, N_IT_BF, xrm16, xim16, la16, "h_",
                         last_keep="skip_mirror")

            # phase boundary: widen bf16 state to fp32 (full mirror range
            # is stale on the last bf16 iter; re-extend after conversion)
            nc.vector.tensor_copy(xrm[:, 0:WBIG], xrm16[:, 0:WBIG])
            nc.vector.tensor_copy(xim[:, 0:WBIG], xim16[:, 0:WBIG])
            nc.scalar.copy(xrm[:, WBIG:], xrm[:, 0:WMIR - WBIG])
            nc.scalar.copy(xim[:, WBIG:], xim[:, 0:WMIR - WBIG])

            with tc.tile_pool(name="dk32", bufs=1) as tp32:
                dk_phase(tp32, F32, N_IT_F32, xrm, xim, laP, "f_",
                         last_keep="skip_mirror")

            # ---- formants: angle, validity, partial sort, normalize ----
            with tc.tile_pool(name="post", bufs=1) as tp:
                xr = xrm[:, 0:WBIG]
                xi = xim[:, 0:WBIG]

                def big(tag, dtype=F32, w=WBIG):
                    return tp.tile([PART, w], dtype, tag=tag, name=tag)

                rx = big("po_rx")
                nc.vector.reciprocal(rx[:], xr)
                tt_ = big("po_t")
                vmul(tt_[:], xi, rx[:])
                nc.vector.tensor_scalar(tt_[:], tt_[:], 1e20, -1e20, Alu.min, Alu.max)
                ang = big("po_ang")
                nc.scalar.activation(ang[:], tt_[:], Act.Arctan)
                neg = big("po_neg", dtype=U8)
                nc.vector.tensor_scalar(neg[:], xr, 0.0, None, Alu.is_lt)
                shifted = big("po_shift")
                nc.vector.tensor_scalar(shifted[:], ang[:], float(PI), None, Alu.add)
                nc.vector.copy_predicated(ang[:], neg[:], shifted[:])

                m1 = big("po_m1", dtype=U8)
                nc.vector.tensor_scalar(m1[:], xi, 0.0, None, Alu.is_gt)
                m2 = big("po_m2", dtype=U8)
                nc.vector.tensor_scalar(m2[:], ang[:], float(ANG_LO), None, Alu.is_gt)
                m3 = big("po_m3", dtype=U8)
                nc.vector.tensor_scalar(m3[:], ang[:], float(ANG_HI), None, Alu.is_lt)
                stt(m1[:], m1[:], m2[:], Alu.logical_and)
                stt(m1[:], m1[:], m3[:], Alu.logical_and)
                angv = big("po_angv")
                nc.vector.memset(angv[:], float(ANG_INVALID))
                nc.vector.copy_predicated(angv[:], m1[:], ang[:])

                # partial selection sort (4 bubble passes over 10 blocks)
                cur = [angv[:, m_ * WCOL:(m_ + 1) * WCOL] for m_ in range(NROOT)]
                for k_ in range(4):
                    for i in range(NROOT - 1, k_, -1):
                        lo = tp.tile([PART, WCOL], F32, tag=f"srt{k_}_{i}a", name=f"srt{k_}_{i}a")
                        hi = tp.tile([PART, WCOL], F32, tag=f"srt{k_}_{i}b", name=f"srt{k_}_{i}b")
                        stt(lo[:], cur[i - 1], cur[i], Alu.min)
                        stt(hi[:], cur[i - 1], cur[i], Alu.max)
                        cur[i - 1] = lo[:]
                        cur[i] = hi[:]
                ot = tp.tile([PART, 4 * WCOL], F32, tag="srt_out", name="srt_out")
                for k_ in range(4):
                    nc.vector.tensor_scalar(
                        ot[:, k_ * WCOL:(k_ + 1) * WCOL], cur[k_],
                        float(OUT_SCALE), -1.0, Alu.mult, Alu.add
                    )
                nc.sync.dma_start(out=out_d[:], in_=ot[:])

    lowp.__exit__(None, None, None)
    _split_multi_waits(nc)
    return nc


# ------------------------------------------------------------- jit runner
_CACHE = {}


def _get_runner():
    if "runner" in _CACHE:
        return _CACHE["runner"]

    import jax
    from jax.sharding import Mesh, PartitionSpec, NamedSharding
    from jax.experimental.shard_map import shard_map
    from concourse import bass2jax
    from concourse import mybir

    nc = _build_module()
    bass2jax.install_neuronx_cc_hook()

    partition_name = nc.partition_id_tensor.name if nc.partition_id_tensor else None
    in_names, out_names, out_avals = [], [], []
    for alloc in nc.m.functions[0].allocations:
        if not isinstance(alloc, mybir.MemoryLocationSet):
            continue
        name = alloc.memorylocations[0].name
        if alloc.kind == "ExternalInput":
            if name != partition_name:
                in_names.append(name)
        elif alloc.kind == "ExternalOutput":
            shape = tuple(alloc.tensor_shape)
            out_names.append(name)
            out_avals.append(jax.core.ShapedArray(shape, mybir.dt.np(alloc.dtype)))
    all_in = in_names + out_names + ([partition_name] if partition_name else [])

    def _body(*args):
        operands = list(args)
        if partition_name:
            operands.append(bass2jax.partition_id_tensor())
        return tuple(bass2jax._bass_exec_p.bind(
            *operands, out_avals=tuple(out_avals), in_names=tuple(all_in),
            out_names=tuple(out_names), lowering_input_output_aliases=(),
            sim_require_finite=True, sim_require_nnan=True, nc=nc))

    mesh = Mesh(np.asarray(jax.devices()[:NCORES]), ("core",))
    n_args = len(in_names) + len(out_names)
    sharded = jax.jit(
        shard_map(_body, mesh=mesh, in_specs=(PartitionSpec("core"),) * n_args,
                  out_specs=(PartitionSpec("core"),) * len(out_names), check_rep=False),
        keep_unused=True)
    sh = NamedSharding(mesh, PartitionSpec("core"))

    # constants + zero output placeholders: device-resident once
    cs, wm = _spec_consts()
    const_dev = {
        "cs": jax.device_put(np.tile(cs, (NCORES, 1)), sh),
        "wm": jax.device_put(np.tile(wm, (NCORES, 1)), sh),
    }
    zeros_dev = [
        jax.device_put(np.zeros((NCORES * a.shape[0], *a.shape[1:]),
                                np.dtype(a.dtype)), sh)
        for a in out_avals
    ]
    jax.block_until_ready(list(const_dev.values()) + zeros_dev)

    runner = {
        "jax": jax, "sharded": sharded, "sh": sh,
        "in_names": in_names, "out_names": out_names, "out_avals": out_avals,
        "const_dev": const_dev, "zeros_dev": zeros_dev,
    }
    _CACHE["runner"] = runner
    return runner


def _prep_kin(r_coeff):
    r_coeff = np.ascontiguousarray(r_coeff, dtype=np.float32)
    kin = np.zeros((NCORES, P, FPAD), np.float32)
    for c in range(NCORES):
        chunk = r_coeff[c * BPC:(c + 1) * BPC]               # (8, 10, 1000)
        kin[c, :, :FPC] = np.transpose(chunk, (1, 0, 2)).reshape(P, FPC)
    # (core, c, p, w) -> (core, p, (c,w)): per-partition contiguous rows
    kin = kin.reshape(NCORES, P, PART, WCOL).transpose(0, 2, 1, 3)
    return np.ascontiguousarray(kin).reshape(NCORES * PART, P * WCOL)


def kernel(r_coeff: np.ndarray) -> np.ndarray:
    import time as _time

    r = _get_runner()
    jax = r["jax"]
    kin = _prep_kin(r_coeff)

    t0 = _time.time()
    args = []
    for n in r["in_names"]:
        args.append(kin if n == "kin" else r["const_dev"][n])
    outs = r["sharded"](*args, *r["zeros_dev"])
    res = np.asarray(outs[0])
    _CACHE["exec_wall_s"] = _time.time() - t0
    _CACHE["last_args"] = args

    res = res.reshape(NCORES, PART, 4, WCOL).transpose(0, 2, 1, 3)
    out = np.empty((B, 4, T), np.float32)
    for c in range(NCORES):
        o = res[c].reshape(4, FPAD)[:, :FPC]
        out[c * BPC:(c + 1) * BPC] = np.transpose(o.reshape(4, BPC, T), (1, 0, 2))
    return out


def _amortized_exec_ns(n=33, trials=3):
    """Estimate on-device exec time per call: dispatch n async calls in one
    flush vs 1; the difference divided by n-1 removes the RTT floor."""
    import time as _time

    r = _get_runner()
    jax = r["jax"]
    args = _CACHE["last_args"]
    dev_args = [a if hasattr(a, "sharding") else jax.device_put(a, r["sh"])
                for a in args]
    jax.block_until_ready(dev_args)

    def flush(k):
        t0 = _time.time()
        rs = [r["sharded"](*dev_args, *r["zeros_dev"]) for _ in range(k)]
        jax.block_until_ready(rs)
        return _time.time() - t0

    flush(1)
    best = None
    for _ in range(trials):
        t1 = min(flush(1) for _ in range(3))
        tN = min(flush(n) for _ in range(2))
        est = (tN - t1) / (n - 1)
        best = est if best is None else min(best, est)
    return int(best * 1e9)
